# revision 3
# baseline (speedup 1.0000x reference)
"""CTC loss (keras ctc_batch_cost semantics) on 8 Trainium2 NeuronCores.

Strategy (pure data parallel, batch sharded 4096 -> 8 x 512):
  - The lattice is split into blank states B_j and label states O_j and
    every alpha is divided by the running product of blank emissions, so
    the blank chain needs no multiply at all and the label chain uses
    host-precomputed emission ratios El_j(t)/Eb(t) (bf16).
  - Forward (alpha) and backward (gamma) recursions run simultaneously,
    meeting in the middle: 128 fused steps instead of 255, each a set of
    five bf16 vector ops over [128, G, 2, 18] tiles (fwd half in natural
    label order, bwd half reversed so both share shift directions).
  - Periodic (every 16 steps) per-chain max-rescale keeps bf16 in range;
    the logs of the scales accumulate in fp32.
  - loss = T*log1p(C*eps) - (log(sum_j T3_j*u_j + sum_j Bhat_j*gBhat_j)
           + logacc_fwd + logacc_bwd + sum_t log Eb_t).
    The blank log-sum is computed on device (ACT Ln + Pool reduce).
"""
import numpy as np

B, T, C, L = 4096, 256, 96, 16
NCORES = 8
BPC = B // NCORES             # 512 batches per core
G = BPC // 128                # 4 groups of 128 on partitions
SS = 18                       # 1 pad slot + 16/17 lattice slots
TP = 128                      # fused fwd/bwd iterations
NCH = 8                       # EL stream chunks
CHT = TP // NCH               # iterations per chunk
RESC = frozenset((15, 31, 47, 63, 79, 95, 111))

_cache = {}


def _build():
    if "nc" in _cache:
        return _cache["nc"]
    import concourse.bacc as bacc
    import concourse.tile as tile
    import concourse.mybir as mybir
    import concourse.bass as bass
    dt = mybir.dt

    nc = bacc.Bacc("TRN2", target_bir_lowering=False, debug=False,
                   enable_asserts=False)
    ROW = G * 2 * SS
    EL_d = nc.dram_tensor("EL", [128, TP * ROW], dt.bfloat16,
                          kind="ExternalInput")
    BLK_d = nc.dram_tensor("BLK", [128, G * T], dt.bfloat16,
                           kind="ExternalInput")
    DM_d = nc.dram_tensor("DM", [128, ROW], dt.bfloat16,
                          kind="ExternalInput")
    IB_d = nc.dram_tensor("IB", [128, G], dt.bfloat16,
                          kind="ExternalInput")
    loss_d = nc.dram_tensor("loss", [BPC, 1], dt.float32,
                            kind="ExternalOutput")
    CNORM = float(T * np.log1p(C * 1e-7))
    Ln = mybir.ActivationFunctionType.Ln
    Copy = mybir.ActivationFunctionType.Copy

    with tile.TileContext(nc) as tc:
        with tc.tile_pool(name="rec", bufs=1) as rec, \
             tc.tile_pool(name="scr", bufs=2) as scr:
            DMt = rec.tile([128, G, 2, SS], dt.bfloat16)
            IBt = rec.tile([128, G], dt.bfloat16)
            BLKt = rec.tile([128, G, T], dt.bfloat16)
            nc.sync.dma_start(DMt[:], DM_d.ap().rearrange(
                "p (g h s) -> p g h s", g=G, h=2))
            nc.sync.dma_start(IBt[:], IB_d.ap())
            nc.scalar.dma_start(BLKt[:], BLK_d.ap().rearrange(
                "p (g t) -> p g t", g=G))
            # emission-ratio stream, one tile per chunk for overlap
            ELd_v = EL_d.ap().rearrange("p (t r) -> p t r", r=ROW)
            ELc = []
            for c in range(NCH):
                t_ = rec.tile([128, CHT, G, 2, SS], dt.bfloat16)
                src = ELd_v[:, c * CHT:(c + 1) * CHT, :]
                dst = t_[:].rearrange("p t g h s -> p t (g h s)")
                (nc.sync if c % 2 == 0 else nc.scalar).dma_start(dst, src)
                ELc.append(t_)

            XOa = rec.tile([128, G, 2, SS], dt.bfloat16)
            XOb = rec.tile([128, G, 2, SS], dt.bfloat16)
            XBa = rec.tile([128, G, 2, SS], dt.bfloat16)
            XBb = rec.tile([128, G, 2, SS], dt.bfloat16)
            logacc = rec.tile([128, G, 2], dt.float32)
            bsum = rec.tile([128, G], dt.float32)
            lnb = rec.tile([128, G, T], dt.float32)
            nc.vector.memset(XOa[:], 0.0)
            nc.vector.memset(XOb[:], 0.0)
            nc.vector.memset(XBa[:], 0.0)
            nc.vector.memset(XBb[:], 0.0)
            nc.vector.memset(logacc[:], 0.0)
            nc.vector.memset(XBa[:, :, :, 1:2], 1.0)   # B_0 = gB_16 = 1
            nc.vector.tensor_copy(XOa[:, :, 1, 1], IBt[:])  # u(255) seed

            # blank log-sum (ACT Ln; one-off DVE reduce)
            nc.scalar.activation(lnb[:], BLKt[:], Ln)
            nc.vector.tensor_reduce(bsum[:], lnb[:], op=mybir.AluOpType.add,
                                    axis=mybir.AxisListType.X)

            cXO, cXB, nXO, nXB = XOa, XBa, XOb, XBb
            t3_last = None
            for i in range(TP):
                ELi = ELc[i // CHT][:, i % CHT]
                t1 = scr.tile([128, G, 2, SS], dt.bfloat16, tag="t1")
                t2 = scr.tile([128, G, 2, SS], dt.bfloat16, tag="t2")
                t3 = scr.tile([128, G, 2, SS], dt.bfloat16, tag="t3")
                nc.vector.tensor_mul(t1[:, :, :, 0:17], cXO[:, :, :, 0:17],
                                     DMt[:, :, :, 0:17])
                nc.vector.tensor_add(t2[:, :, :, 1:17], cXO[:, :, :, 1:17],
                                     cXB[:, :, :, 1:17])
                nc.vector.tensor_add(t3[:, :, :, 1:17], t2[:, :, :, 1:17],
                                     t1[:, :, :, 0:16])
                nc.vector.tensor_add(nXB[:, :, :, 1:18], cXB[:, :, :, 1:18],
                                     cXO[:, :, :, 0:17])
                if i < TP - 1:
                    nc.vector.tensor_mul(nXO[:, :, :, 1:17], t3[:, :, :, 1:17],
                                         ELi[:, :, :, 1:17])
                else:
                    nc.vector.tensor_mul(nXO[:, :, 1, 1:17], t3[:, :, 1, 1:17],
                                         ELi[:, :, 1, 1:17])
                    t3_last = t3
                cXO, nXO = nXO, cXO
                cXB, nXB = nXB, cXB
                if i in RESC:
                    rmx = scr.tile([128, G, 2, 1], dt.float32, tag="rmx")
                    m2 = scr.tile([128, G, 2, 1], dt.float32, tag="m2")
                    nc.vector.tensor_reduce(rmx[:, :, :, 0], cXO[:],
                                            op=mybir.AluOpType.max,
                                            axis=mybir.AxisListType.X)
                    nc.vector.tensor_reduce(m2[:, :, :, 0], cXB[:],
                                            op=mybir.AluOpType.max,
                                            axis=mybir.AxisListType.X)
                    nc.vector.tensor_max(rmx[:], rmx[:], m2[:])
                    rl = scr.tile([128, G, 2], dt.float32, tag="rl")
                    nc.scalar.activation(rl[:], rmx[:, :, :, 0], Ln)
                    nc.vector.tensor_add(logacc[:], logacc[:], rl[:])
                    rinv = scr.tile([128, G, 2, 1], dt.float32, tag="ri")
                    nc.vector.reciprocal(rinv[:], rmx[:])
                    rb = rinv[:].broadcast_to((128, G, 2, SS))
                    nc.vector.tensor_mul(cXO[:], cXO[:], rb)
                    nc.vector.tensor_mul(cXB[:], cXB[:], rb)

            # combine: fwd meets bwd (bwd half stored label-reversed)
            S = scr.tile([128, G, 33], dt.float32, tag="S")
            nc.vector.tensor_mul(S[:, :, 0:16], t3_last[:, :, 0, 16:0:-1],
                                 cXO[:, :, 1, 1:17])
            nc.vector.tensor_mul(S[:, :, 16:33], cXB[:, :, 0, 17:0:-1],
                                 cXB[:, :, 1, 1:18])
            Rt = scr.tile([128, G], dt.float32, tag="R")
            nc.vector.tensor_reduce(Rt[:], S[:], op=mybir.AluOpType.add,
                                    axis=mybir.AxisListType.X)
            lnR = scr.tile([128, G], dt.float32, tag="lnR")
            nc.scalar.activation(lnR[:], Rt[:], Ln)
            tot = scr.tile([128, G], dt.float32, tag="tot")
            nc.vector.tensor_add(tot[:], lnR[:], logacc[:, :, 0])
            nc.vector.tensor_add(tot[:], tot[:], logacc[:, :, 1])
            nc.vector.tensor_add(tot[:], tot[:], bsum[:])
            res = scr.tile([128, G], dt.float32, tag="res")
            nc.scalar.activation(res[:], tot[:], Copy, bias=CNORM, scale=-1.0)
            out_ap = bass.AP(loss_d.ap().tensor, 0, [[1, 128], [128, G]])
            nc.sync.dma_start(out_ap, res[:])

    nc.compile()
    _cache["nc"] = nc
    return nc


def _host_core(y, lab, c):
    """Inputs for core c: emission ratios, blanks, skip masks, bwd seed."""
    import ml_dtypes
    bf = ml_dtypes.bfloat16
    sl = slice(c * BPC, (c + 1) * BPC)
    yc = y[sl]
    labc = lab[sl].astype(np.int64)
    blank = yc[:, :, C - 1]                                  # [BPC,T]
    bs = np.arange(BPC)
    el = yc[bs[:, None, None], np.arange(T)[None, :, None],
            labc[:, None, :]]                                # [BPC,T,L]
    ratio = (el / blank[:, :, None]).astype(np.float32)
    r4 = ratio.reshape(G, 128, T, L)
    ELh = np.zeros((128, TP, G, 2, SS), np.float32)
    # fwd: El_j(t) at slot 1+j, iterations 0..126
    ELh[:, 0:127, :, 0, 1:17] = r4[:, :, 0:127, :].transpose(1, 2, 0, 3)
    # bwd: El_{15-k}(254-i) at slot 1+k, iterations 0..127
    rr4 = ratio[:, ::-1, ::-1].reshape(G, 128, T, L)
    ELh[:, :, :, 1, 1:17] = rr4[:, :, 1:TP + 1, :].transpose(1, 2, 0, 3)
    BLKh = blank.reshape(G, 128, T).transpose(1, 0, 2)
    dd = np.zeros((BPC, L), np.float32)
    dd[:, 1:] = labc[:, 1:] != labc[:, :-1]
    DMh = np.zeros((BPC, 2, SS), np.float32)
    DMh[:, 0, 1:16] = dd[:, 1:16]            # fwd mask d_s at slot s
    DMh[:, 1, 1:16] = dd[:, 15:0:-1]         # bwd mask d_{16-s} at slot s
    DMh = DMh.reshape(G, 128, 2, SS).transpose(1, 0, 2, 3)
    IBh = ratio[:, T - 1, L - 1].reshape(G, 128).transpose(1, 0)
    return {
        "EL": np.ascontiguousarray(ELh.reshape(128, TP * G * 2 * SS)).astype(bf),
        "BLK": np.ascontiguousarray(BLKh.reshape(128, G * T)).astype(bf),
        "DM": np.ascontiguousarray(DMh.reshape(128, G * 2 * SS)).astype(bf),
        "IB": np.ascontiguousarray(IBh).astype(bf),
    }


def _fallback(y_pred, labels, input_length, label_length):
    """Exact log-domain numpy replica of the reference (generic lengths)."""
    y = np.asarray(y_pred, np.float32)
    lab = np.asarray(labels).astype(np.int64)
    il = np.asarray(input_length)[:, 0].astype(np.int64)
    ll = np.asarray(label_length)[:, 0].astype(np.int64)
    Bn, Tn, Cn = y.shape
    Ln = lab.shape[1]
    Sn = 2 * Ln + 1
    NEG = np.float32(-1e30)
    logp = np.log(y + 1e-7, dtype=np.float32)
    logp = logp - np.log(np.sum(np.exp(logp - logp.max(-1, keepdims=True)),
                                -1, keepdims=True)) - logp.max(-1, keepdims=True)
    ext = np.full((Bn, Sn), Cn - 1, np.int64)
    ext[:, 1::2] = lab
    sidx = np.arange(Sn)
    state_valid = sidx[None, :] < (2 * ll[:, None] + 1)
    skip = np.zeros((Bn, Sn), bool)
    skip[:, 3::2] = ext[:, 3::2] != ext[:, 1:-2:2]
    emit = logp[np.arange(Bn)[:, None, None], np.arange(Tn)[None, :, None],
                ext[:, None, :]]                      # [B,T,S]
    alpha = np.full((Bn, Sn), NEG, np.float32)
    alpha[:, 0] = emit[:, 0, 0]
    alpha[:, 1] = np.where(ll >= 1, emit[:, 0, 1], NEG)

    def lae(a, b):
        m = np.maximum(a, b)
        return m + np.log1p(np.exp(-np.abs(a - b)))
    for t in range(1, Tn):
        p1 = np.concatenate([np.full((Bn, 1), NEG), alpha[:, :-1]], 1)
        p2 = np.concatenate([np.full((Bn, 2), NEG), alpha[:, :-2]], 1)
        p2 = np.where(skip, p2, NEG)
        new = lae(lae(alpha, p1), p2) + emit[:, t, :]
        new = np.where(state_valid, new, NEG)
        alpha = np.where((t < il)[:, None], new, alpha)
    bi = np.arange(Bn)
    a_b = alpha[bi, 2 * ll]
    a_l = alpha[bi, np.maximum(2 * ll - 1, 0)]
    logp_f = np.where(ll > 0, lae(a_b, a_l), a_b)
    return (-logp_f[:, None]).astype(np.float32)


def _run(y_pred, labels, trace=False):
    from concourse import bass_utils
    nc = _build()
    y = np.asarray(y_pred, np.float32)
    lab = np.asarray(labels)
    in_maps = [_host_core(y, lab, c) for c in range(NCORES)]
    res = bass_utils.run_bass_kernel_spmd(nc, in_maps,
                                          core_ids=list(range(NCORES)),
                                          trace=trace)
    out = np.concatenate([res.results[c]["loss"] for c in range(NCORES)], 0)
    return out.astype(np.float32), res


def kernel(y_pred, labels, input_length, label_length):
    y_pred = np.ascontiguousarray(np.asarray(y_pred, np.float32))
    labels = np.asarray(labels)
    il = np.asarray(input_length)
    ll = np.asarray(label_length)
    if (y_pred.shape != (B, T, C) or labels.shape != (B, L)
            or not np.all(il == T) or not np.all(ll == L)):
        return _fallback(y_pred, labels, il, ll)

    try:
        out, _ = _run(y_pred, labels)
        return out
    except Exception:
        return _fallback(y_pred, labels, il, ll)


# revision 8
# speedup vs baseline: 1.0070x; 1.0070x over previous
"""CTC loss (keras ctc_batch_cost semantics) on 8 Trainium2 NeuronCores.

Strategy (pure data parallel, batch sharded 4096 -> 8 x 512):
  - The lattice is split into blank states B_j and label states O_j and
    every alpha is divided by the running product of blank emissions, so
    the blank chain needs no multiply at all and the label chain uses
    host-precomputed emission ratios El_j(t)/Eb(t) (bf16).
  - Forward (alpha) and backward (gamma) recursions run simultaneously,
    meeting in the middle: 128 fused steps instead of 255, each a set of
    five bf16 vector ops over [128, G, 2, 18] tiles (fwd half in natural
    label order, bwd half reversed so both share shift directions).
  - Periodic (every 16 steps) per-chain max-rescale keeps bf16 in range;
    the logs of the scales accumulate in fp32.
  - loss = T*log1p(C*eps) - (log(sum_j T3_j*u_j + sum_j Bhat_j*gBhat_j)
           + logacc_fwd + logacc_bwd + sum_t log Eb_t).
    The blank log-sum is computed on device (ACT Ln + Pool reduce).
"""
import numpy as np

B, T, C, L = 4096, 256, 96, 16
NCORES = 8
BPC = B // NCORES             # 512 batches per core
G = BPC // 128                # 4 groups of 128 on partitions
SS = 18                       # 1 pad slot + 16/17 lattice slots
TP = 128                      # fused fwd/bwd iterations
NCH = 8                       # EL stream chunks
CHT = TP // NCH               # iterations per chunk
RESC = frozenset((31, 63, 95, 111))
NRESC = len(RESC)
LN2 = 0.6931471805599453
# fp32/bf16 exponent extractions per output element: 2 chains x NRESC
# rescales + 1 final combine + T blank factors; each carries a +127 bias.
KBIAS = 127.0 * (2 * NRESC + 1 + T)

_cache = {}


def _build():
    if "nc" in _cache:
        return _cache["nc"]
    import concourse.bacc as bacc
    import concourse.tile as tile
    import concourse.mybir as mybir
    import concourse.bass as bass
    dt = mybir.dt

    nc = bacc.Bacc("TRN2", target_bir_lowering=False, debug=False,
                   enable_asserts=False)
    ROW = G * 2 * SS
    EL_d = nc.dram_tensor("EL", [128, TP * ROW], dt.bfloat16,
                          kind="ExternalInput")
    BLK_d = nc.dram_tensor("BLK", [128, G * T], dt.bfloat16,
                           kind="ExternalInput")
    DM_d = nc.dram_tensor("DM", [128, ROW], dt.bfloat16,
                          kind="ExternalInput")
    IB_d = nc.dram_tensor("IB", [128, G], dt.bfloat16,
                          kind="ExternalInput")
    loss_d = nc.dram_tensor("loss", [BPC, 1], dt.float32,
                            kind="ExternalOutput")
    CNORM = float(T * np.log1p(C * 1e-7))
    Ln = mybir.ActivationFunctionType.Ln
    Copy = mybir.ActivationFunctionType.Copy

    with tile.TileContext(nc) as tc:
        with tc.tile_pool(name="rec", bufs=1) as rec, \
             tc.tile_pool(name="scr", bufs=2) as scr:
            DMt = rec.tile([128, G, 2, SS], dt.bfloat16)
            IBt = rec.tile([128, G], dt.bfloat16)
            BLKt = rec.tile([128, G, T], dt.bfloat16)
            nc.sync.dma_start(DMt[:], DM_d.ap().rearrange(
                "p (g h s) -> p g h s", g=G, h=2))
            nc.sync.dma_start(IBt[:], IB_d.ap())
            nc.scalar.dma_start(BLKt[:], BLK_d.ap().rearrange(
                "p (g t) -> p g t", g=G))
            # emission-ratio stream, one tile per chunk for overlap
            ELd_v = EL_d.ap().rearrange("p (t r) -> p t r", r=ROW)
            ELc = []
            for c in range(NCH):
                t_ = rec.tile([128, CHT, G, 2, SS], dt.bfloat16)
                src = ELd_v[:, c * CHT:(c + 1) * CHT, :]
                dst = t_[:].rearrange("p t g h s -> p t (g h s)")
                (nc.sync if c % 2 == 0 else nc.scalar).dma_start(dst, src)
                ELc.append(t_)

            XOa = rec.tile([128, G, 2, SS], dt.bfloat16)
            XOb = rec.tile([128, G, 2, SS], dt.bfloat16)
            XBa = rec.tile([128, G, 2, SS], dt.bfloat16)
            XBb = rec.tile([128, G, 2, SS], dt.bfloat16)
            logacc = rec.tile([128, G, 2], dt.float32)
            bsum = rec.tile([128, G], dt.float32)
            lnb = rec.tile([128, G, T], dt.float32)
            nc.vector.memset(XOa[:], 0.0)
            nc.vector.memset(XOb[:], 0.0)
            nc.vector.memset(XBa[:], 0.0)
            nc.vector.memset(XBb[:], 0.0)
            nc.vector.memset(logacc[:], 0.0)
            nc.vector.memset(XBa[:, :, :, 1:2], 1.0)   # B_0 = gB_16 = 1
            nc.vector.tensor_copy(XOa[:, :, 1, 1], IBt[:])  # u(255) seed

            # blank log-sum: split each bf16 into exponent (exact int ops on
            # Pool) and mantissa in [1,2) (table-safe ACT Ln), reduce on DVE.
            bm = rec.tile([128, G, T], dt.bfloat16)
            bku = rec.tile([128, G, T], dt.uint16)
            bk32 = rec.tile([128, G, T], dt.float32)
            bksum = rec.tile([128, G], dt.float32)
            bb = BLKt[:].bitcast(dt.uint16)
            nc.vector.tensor_scalar(bm[:].bitcast(dt.uint16), bb,
                                    0x007F, 0x3F80,
                                    op0=mybir.AluOpType.bitwise_and,
                                    op1=mybir.AluOpType.bitwise_or)
            nc.vector.tensor_scalar(bku[:], bb, 7, None,
                                    op0=mybir.AluOpType.logical_shift_right)
            nc.vector.tensor_copy(bk32[:], bku[:])
            for g in range(G):
                nc.scalar.activation(lnb[:, g], bm[:, g], Ln,
                                     accum_out=bsum[:, g:g + 1])
            nc.vector.tensor_reduce(bksum[:], bk32[:], op=mybir.AluOpType.add,
                                    axis=mybir.AxisListType.X)

            cXO, cXB, nXO, nXB = XOa, XBa, XOb, XBb
            t3_last = None
            for i in range(TP):
                ELi = ELc[i // CHT][:, i % CHT]
                t1 = scr.tile([128, G, 2, SS], dt.bfloat16, tag="t1")
                t2 = scr.tile([128, G, 2, SS], dt.bfloat16, tag="t2")
                t3 = scr.tile([128, G, 2, SS], dt.bfloat16, tag="t3")
                nc.vector.tensor_mul(t1[:, :, :, 0:17], cXO[:, :, :, 0:17],
                                     DMt[:, :, :, 0:17])
                nc.vector.tensor_add(t2[:, :, :, 1:17], cXO[:, :, :, 1:17],
                                     cXB[:, :, :, 1:17])
                nc.vector.tensor_add(t3[:, :, :, 1:17], t2[:, :, :, 1:17],
                                     t1[:, :, :, 0:16])
                nc.vector.tensor_add(nXB[:, :, :, 1:18], cXB[:, :, :, 1:18],
                                     cXO[:, :, :, 0:17])
                if i < TP - 1:
                    nc.vector.tensor_mul(nXO[:, :, :, 1:17], t3[:, :, :, 1:17],
                                         ELi[:, :, :, 1:17])
                else:
                    nc.vector.tensor_mul(nXO[:, :, 1, 1:17], t3[:, :, 1, 1:17],
                                         ELi[:, :, 1, 1:17])
                    t3_last = t3
                cXO, nXO = nXO, cXO
                cXB, nXB = nXB, cXB
                if i in RESC:
                    rmx = scr.tile([128, G, 2, 1], dt.float32, tag="rmx")
                    m2 = scr.tile([128, G, 2, 1], dt.float32, tag="m2")
                    nc.vector.tensor_reduce(rmx[:, :, :, 0], cXO[:],
                                            op=mybir.AluOpType.max,
                                            axis=mybir.AxisListType.X)
                    nc.vector.tensor_reduce(m2[:, :, :, 0], cXB[:],
                                            op=mybir.AluOpType.max,
                                            axis=mybir.AxisListType.X)
                    nc.vector.tensor_max(rmx[:], rmx[:], m2[:])
                    # rescale by 2^floor(log2 max): exact, no Ln needed
                    rbits = rmx[:].bitcast(dt.uint32)
                    ku = scr.tile([128, G, 2, 1], dt.uint32, tag="ku")
                    nc.vector.tensor_scalar(
                        ku[:], rbits, 23, None,
                        op0=mybir.AluOpType.logical_shift_right)
                    kf = scr.tile([128, G, 2], dt.float32, tag="kf")
                    nc.vector.tensor_copy(kf[:], ku[:, :, :, 0])
                    nc.vector.tensor_add(logacc[:], logacc[:], kf[:])
                    eb = scr.tile([128, G, 2, 1], dt.uint32, tag="eb")
                    nc.vector.tensor_scalar(
                        eb[:], rbits, 0x7F800000, None,
                        op0=mybir.AluOpType.bitwise_and)
                    rinv = scr.tile([128, G, 2, 1], dt.float32, tag="ri")
                    nc.vector.reciprocal(rinv[:], eb[:].bitcast(dt.float32))
                    rb = rinv[:].broadcast_to((128, G, 2, SS))
                    nc.vector.tensor_mul(cXO[:], cXO[:], rb)
                    nc.vector.tensor_mul(cXB[:], cXB[:], rb)

            # combine: fwd meets bwd (bwd half stored label-reversed)
            S = scr.tile([128, G, 33], dt.float32, tag="S")
            nc.vector.tensor_mul(S[:, :, 0:16], t3_last[:, :, 0, 16:0:-1],
                                 cXO[:, :, 1, 1:17])
            nc.vector.tensor_mul(S[:, :, 16:33], cXB[:, :, 0, 17:0:-1],
                                 cXB[:, :, 1, 1:18])
            Rt = scr.tile([128, G], dt.float32, tag="R")
            nc.vector.tensor_reduce(Rt[:], S[:], op=mybir.AluOpType.add,
                                    axis=mybir.AxisListType.X)
            # ln R = ln(mantissa) + k*ln2, with ln on [1,2) only
            Rb = Rt[:].bitcast(dt.uint32)
            kRu = scr.tile([128, G], dt.uint32, tag="kRu")
            nc.vector.tensor_scalar(kRu[:], Rb, 23, None,
                                    op0=mybir.AluOpType.logical_shift_right)
            kR = scr.tile([128, G], dt.float32, tag="kR")
            nc.vector.tensor_copy(kR[:], kRu[:])
            eR = scr.tile([128, G], dt.uint32, tag="eR")
            nc.vector.tensor_scalar(eR[:], Rb, 0x7F800000, None,
                                    op0=mybir.AluOpType.bitwise_and)
            riR = scr.tile([128, G], dt.float32, tag="riR")
            nc.vector.reciprocal(riR[:], eR[:].bitcast(dt.float32))
            mR = scr.tile([128, G], dt.float32, tag="mR")
            nc.vector.tensor_mul(mR[:], Rt[:], riR[:])
            lnR = scr.tile([128, G], dt.float32, tag="lnR")
            nc.scalar.activation(lnR[:], mR[:], Ln)
            # K = sum of all (k+127) exponents (exact integers in fp32)
            K = scr.tile([128, G], dt.float32, tag="K")
            nc.vector.tensor_add(K[:], kR[:], logacc[:, :, 0])
            nc.vector.tensor_add(K[:], K[:], logacc[:, :, 1])
            nc.vector.tensor_add(K[:], K[:], bksum[:])
            tot = scr.tile([128, G], dt.float32, tag="tot")
            nc.vector.tensor_scalar(tot[:], K[:], -KBIAS, LN2,
                                    op0=mybir.AluOpType.add,
                                    op1=mybir.AluOpType.mult)
            nc.vector.tensor_add(tot[:], tot[:], lnR[:])
            nc.vector.tensor_add(tot[:], tot[:], bsum[:])
            res = scr.tile([128, G], dt.float32, tag="res")
            nc.scalar.activation(res[:], tot[:], Copy, bias=CNORM, scale=-1.0)
            out_ap = bass.AP(loss_d.ap().tensor, 0, [[1, 128], [128, G]])
            nc.sync.dma_start(out_ap, res[:])

    nc.compile()
    _cache["nc"] = nc
    return nc


def _host_core(y, lab, c):
    """Inputs for core c: emission ratios, blanks, skip masks, bwd seed."""
    import ml_dtypes
    bf = ml_dtypes.bfloat16
    sl = slice(c * BPC, (c + 1) * BPC)
    yc = y[sl]
    labc = lab[sl].astype(np.int64)
    blank = yc[:, :, C - 1]                                  # [BPC,T]
    bs = np.arange(BPC)
    el = yc[bs[:, None, None], np.arange(T)[None, :, None],
            labc[:, None, :]]                                # [BPC,T,L]
    ratio = (el / blank[:, :, None]).astype(np.float32)
    r4 = ratio.reshape(G, 128, T, L)
    ELh = np.zeros((128, TP, G, 2, SS), np.float32)
    # fwd: El_j(t) at slot 1+j, iterations 0..126
    ELh[:, 0:127, :, 0, 1:17] = r4[:, :, 0:127, :].transpose(1, 2, 0, 3)
    # bwd: El_{15-k}(254-i) at slot 1+k, iterations 0..127
    rr4 = ratio[:, ::-1, ::-1].reshape(G, 128, T, L)
    ELh[:, :, :, 1, 1:17] = rr4[:, :, 1:TP + 1, :].transpose(1, 2, 0, 3)
    BLKh = blank.reshape(G, 128, T).transpose(1, 0, 2)
    dd = np.zeros((BPC, L), np.float32)
    dd[:, 1:] = labc[:, 1:] != labc[:, :-1]
    DMh = np.zeros((BPC, 2, SS), np.float32)
    DMh[:, 0, 1:16] = dd[:, 1:16]            # fwd mask d_s at slot s
    DMh[:, 1, 1:16] = dd[:, 15:0:-1]         # bwd mask d_{16-s} at slot s
    DMh = DMh.reshape(G, 128, 2, SS).transpose(1, 0, 2, 3)
    IBh = ratio[:, T - 1, L - 1].reshape(G, 128).transpose(1, 0)
    return {
        "EL": np.ascontiguousarray(ELh.reshape(128, TP * G * 2 * SS)).astype(bf),
        "BLK": np.ascontiguousarray(BLKh.reshape(128, G * T)).astype(bf),
        "DM": np.ascontiguousarray(DMh.reshape(128, G * 2 * SS)).astype(bf),
        "IB": np.ascontiguousarray(IBh).astype(bf),
    }


def _fallback(y_pred, labels, input_length, label_length):
    """Exact log-domain numpy replica of the reference (generic lengths)."""
    y = np.asarray(y_pred, np.float32)
    lab = np.asarray(labels).astype(np.int64)
    il = np.asarray(input_length)[:, 0].astype(np.int64)
    ll = np.asarray(label_length)[:, 0].astype(np.int64)
    Bn, Tn, Cn = y.shape
    Ln = lab.shape[1]
    Sn = 2 * Ln + 1
    NEG = np.float32(-1e30)
    logp = np.log(y + 1e-7, dtype=np.float32)
    logp = logp - np.log(np.sum(np.exp(logp - logp.max(-1, keepdims=True)),
                                -1, keepdims=True)) - logp.max(-1, keepdims=True)
    ext = np.full((Bn, Sn), Cn - 1, np.int64)
    ext[:, 1::2] = lab
    sidx = np.arange(Sn)
    state_valid = sidx[None, :] < (2 * ll[:, None] + 1)
    skip = np.zeros((Bn, Sn), bool)
    skip[:, 3::2] = ext[:, 3::2] != ext[:, 1:-2:2]
    emit = logp[np.arange(Bn)[:, None, None], np.arange(Tn)[None, :, None],
                ext[:, None, :]]                      # [B,T,S]
    alpha = np.full((Bn, Sn), NEG, np.float32)
    alpha[:, 0] = emit[:, 0, 0]
    alpha[:, 1] = np.where(ll >= 1, emit[:, 0, 1], NEG)

    def lae(a, b):
        m = np.maximum(a, b)
        return m + np.log1p(np.exp(-np.abs(a - b)))
    for t in range(1, Tn):
        p1 = np.concatenate([np.full((Bn, 1), NEG), alpha[:, :-1]], 1)
        p2 = np.concatenate([np.full((Bn, 2), NEG), alpha[:, :-2]], 1)
        p2 = np.where(skip, p2, NEG)
        new = lae(lae(alpha, p1), p2) + emit[:, t, :]
        new = np.where(state_valid, new, NEG)
        alpha = np.where((t < il)[:, None], new, alpha)
    bi = np.arange(Bn)
    a_b = alpha[bi, 2 * ll]
    a_l = alpha[bi, np.maximum(2 * ll - 1, 0)]
    logp_f = np.where(ll > 0, lae(a_b, a_l), a_b)
    return (-logp_f[:, None]).astype(np.float32)


def _run(y_pred, labels, trace=False):
    from concourse import bass_utils
    nc = _build()
    y = np.asarray(y_pred, np.float32)
    lab = np.asarray(labels)
    in_maps = [_host_core(y, lab, c) for c in range(NCORES)]
    res = bass_utils.run_bass_kernel_spmd(nc, in_maps,
                                          core_ids=list(range(NCORES)),
                                          trace=trace)
    out = np.concatenate([res.results[c]["loss"] for c in range(NCORES)], 0)
    return out.astype(np.float32), res


def kernel(y_pred, labels, input_length, label_length):
    y_pred = np.ascontiguousarray(np.asarray(y_pred, np.float32))
    labels = np.asarray(labels)
    il = np.asarray(input_length)
    ll = np.asarray(label_length)
    if (y_pred.shape != (B, T, C) or labels.shape != (B, L)
            or not np.all(il == T) or not np.all(ll == L)):
        return _fallback(y_pred, labels, il, ll)

    try:
        out, _ = _run(y_pred, labels)
        return out
    except Exception:
        return _fallback(y_pred, labels, il, ll)


# revision 9
# speedup vs baseline: 1.1476x; 1.1396x over previous
"""CTC loss (keras ctc_batch_cost semantics) on 8 Trainium2 NeuronCores.

Strategy (pure data parallel, batch sharded 4096 -> 8 x 512):
  - The lattice is split into blank states B_j and label states O_j and
    every alpha is divided by the running product of blank emissions, so
    the blank chain needs no multiply at all and the label chain uses
    host-precomputed emission ratios El_j(t)/Eb(t) (bf16).
  - Forward (alpha) and backward (gamma) recursions run simultaneously,
    meeting in the middle: 128 fused steps instead of 255.  The 8
    independent chains (4 groups x fwd/bwd) are interleaved as the
    innermost stride-1 dim of [128, 18, 8] tiles, so each of the five
    bf16 vector ops per step is one contiguous aligned run; the bwd
    half is stored label-reversed so both directions share shifts.
  - Every 32 steps each chain is rescaled by 2^floor(log2 max) —
    exponent extracted with integer bit ops, so no wide-range Ln is
    ever evaluated; the exponents accumulate exactly in fp32.
  - ln() only ever sees mantissas in [1,2): the blank log-sum and the
    final combine split values into exponent + mantissa the same way.
  - loss = T*log1p(C*eps) - (ln R + ln2*(sum of all exponents - bias)
           + sum_t ln mant(Eb_t)).
"""
import numpy as np

B, T, C, L = 4096, 256, 96, 16
NCORES = 8
BPC = B // NCORES             # 512 batches per core
G = BPC // 128                # 4 groups of 128 on partitions
NB = 2 * G                    # interleaved chains: block b = 2g + (0 fwd | 1 bwd)
SS = 18                       # 1 pad slot + 16/17 lattice slots
TP = 128                      # fused fwd/bwd iterations
NCH = 8                       # EL stream chunks
CHT = TP // NCH               # iterations per chunk
RESC = frozenset((31, 63, 95, 111))
NRESC = len(RESC)
LN2 = 0.6931471805599453
# fp32/bf16 exponent extractions per output element: 2 chains x NRESC
# rescales + 1 final combine + T blank factors; each carries a +127 bias.
KBIAS = 127.0 * (2 * NRESC + 1 + T)

_cache = {}


def _build():
    if "nc" in _cache:
        return _cache["nc"]
    import concourse.bacc as bacc
    import concourse.tile as tile
    import concourse.mybir as mybir
    import concourse.bass as bass
    dt = mybir.dt

    nc = bacc.Bacc("TRN2", target_bir_lowering=False, debug=False,
                   enable_asserts=False)
    ROW = SS * NB
    EL_d = nc.dram_tensor("EL", [128, TP * ROW], dt.bfloat16,
                          kind="ExternalInput")
    BLK_d = nc.dram_tensor("BLK", [128, G * T], dt.bfloat16,
                           kind="ExternalInput")
    DM_d = nc.dram_tensor("DM", [128, ROW], dt.bfloat16,
                          kind="ExternalInput")
    IB_d = nc.dram_tensor("IB", [128, G], dt.bfloat16,
                          kind="ExternalInput")
    loss_d = nc.dram_tensor("loss", [BPC, 1], dt.float32,
                            kind="ExternalOutput")
    CNORM = float(T * np.log1p(C * 1e-7))
    Ln = mybir.ActivationFunctionType.Ln
    Copy = mybir.ActivationFunctionType.Copy
    AND = mybir.AluOpType.bitwise_and
    OR = mybir.AluOpType.bitwise_or
    SHR = mybir.AluOpType.logical_shift_right

    with tile.TileContext(nc) as tc:
        with tc.tile_pool(name="rec", bufs=1) as rec, \
             tc.tile_pool(name="scr", bufs=2) as scr:
            DMt = rec.tile([128, SS, NB], dt.bfloat16)
            IBt = rec.tile([128, G], dt.bfloat16)
            BLKt = rec.tile([128, G, T], dt.bfloat16)
            nc.sync.dma_start(DMt[:], DM_d.ap().rearrange(
                "p (s b) -> p s b", s=SS))
            nc.sync.dma_start(IBt[:], IB_d.ap())
            nc.scalar.dma_start(BLKt[:], BLK_d.ap().rearrange(
                "p (g t) -> p g t", g=G))
            # emission-ratio stream; one tile per chunk so loads prefetch
            ELd_v = EL_d.ap().rearrange("p (t r) -> p t r", r=ROW)
            ELc = []
            for c in range(NCH):
                elt = rec.tile([128, CHT, SS, NB], dt.bfloat16,
                               tag=f"el{c}", name=f"el{c}")
                src = ELd_v[:, c * CHT:(c + 1) * CHT, :]
                dst = elt[:].rearrange("p t s b -> p t (s b)")
                (nc.sync if c % 2 == 0 else nc.scalar).dma_start(dst, src)
                ELc.append(elt)

            XOa = rec.tile([128, SS, NB], dt.bfloat16)
            XOb = rec.tile([128, SS, NB], dt.bfloat16)
            XBa = rec.tile([128, SS, NB], dt.bfloat16)
            XBb = rec.tile([128, SS, NB], dt.bfloat16)
            logacc = rec.tile([128, NB], dt.float32)
            nc.vector.memset(XOa[:], 0.0)
            nc.vector.memset(XOb[:], 0.0)
            nc.vector.memset(XBa[:], 0.0)
            nc.vector.memset(XBb[:], 0.0)
            nc.vector.memset(logacc[:], 0.0)
            nc.vector.memset(XBa[:, 1:2, :], 1.0)     # B_0 = gB_16 = 1
            nc.vector.tensor_copy(XOa[:, 1, 1::2], IBt[:])  # u(255) seed

            cXO, cXB, nXO, nXB = XOa, XBa, XOb, XBb
            t3_last = None
            for i in range(TP):
                ELi = ELc[i // CHT][:, i % CHT]
                t1 = scr.tile([128, SS, NB], dt.bfloat16, tag="t1")
                t2 = scr.tile([128, SS, NB], dt.bfloat16, tag="t2")
                t3 = scr.tile([128, SS, NB], dt.bfloat16, tag="t3")
                nc.vector.tensor_mul(t1[:, 0:17], cXO[:, 0:17], DMt[:, 0:17])
                nc.vector.tensor_add(t2[:, 1:17], cXO[:, 1:17], cXB[:, 1:17])
                nc.vector.tensor_add(t3[:, 1:17], t2[:, 1:17], t1[:, 0:16])
                nc.vector.tensor_add(nXB[:, 1:18], cXB[:, 1:18], cXO[:, 0:17])
                if i < TP - 1:
                    nc.vector.tensor_mul(nXO[:, 1:17], t3[:, 1:17],
                                         ELi[:, 1:17])
                else:
                    nc.vector.tensor_mul(nXO[:, 1:17, 1::2],
                                         t3[:, 1:17, 1::2],
                                         ELi[:, 1:17, 1::2])
                    t3_last = t3
                cXO, nXO = nXO, cXO
                cXB, nXB = nXB, cXB
                if i in RESC:
                    rmx = scr.tile([128, 1, NB], dt.float32, tag="rmx")
                    m2 = scr.tile([128, 1, NB], dt.float32, tag="m2")
                    nc.vector.tensor_reduce(rmx[:, 0, :],
                                            cXO[:].transpose([0, 2, 1]),
                                            op=mybir.AluOpType.max,
                                            axis=mybir.AxisListType.X)
                    nc.vector.tensor_reduce(m2[:, 0, :],
                                            cXB[:].transpose([0, 2, 1]),
                                            op=mybir.AluOpType.max,
                                            axis=mybir.AxisListType.X)
                    nc.vector.tensor_max(rmx[:], rmx[:], m2[:])
                    # rescale by 2^floor(log2 max): exact, no Ln needed
                    rbits = rmx[:].bitcast(dt.uint32)
                    ku = scr.tile([128, 1, NB], dt.uint32, tag="ku")
                    nc.vector.tensor_scalar(ku[:], rbits, 23, None, op0=SHR)
                    kf = scr.tile([128, NB], dt.float32, tag="kf")
                    nc.vector.tensor_copy(kf[:], ku[:, 0, :])
                    nc.vector.tensor_add(logacc[:], logacc[:], kf[:])
                    eb = scr.tile([128, 1, NB], dt.uint32, tag="eb")
                    nc.vector.tensor_scalar(eb[:], rbits, 0x7F800000, None,
                                            op0=AND)
                    rinv = scr.tile([128, 1, NB], dt.float32, tag="ri")
                    nc.vector.reciprocal(rinv[:], eb[:].bitcast(dt.float32))
                    rb = rinv[:].broadcast_to((128, SS, NB))
                    nc.vector.tensor_mul(cXO[:], cXO[:], rb)
                    nc.vector.tensor_mul(cXB[:], cXB[:], rb)

            # combine: fwd (even blocks) meets bwd (odd blocks, reversed)
            S = scr.tile([128, 33, G], dt.float32, tag="S")
            nc.vector.tensor_mul(S[:, 0:16], t3_last[:, 16:0:-1, 0::2],
                                 cXO[:, 1:17, 1::2])
            nc.vector.tensor_mul(S[:, 16:33], cXB[:, 17:0:-1, 0::2],
                                 cXB[:, 1:18, 1::2])
            Rt = scr.tile([128, G], dt.float32, tag="R")
            nc.vector.tensor_reduce(Rt[:], S[:].transpose([0, 2, 1]),
                                    op=mybir.AluOpType.add,
                                    axis=mybir.AxisListType.X)

            # blank log-sum: exponents via exact int ops, Ln on mantissa
            bm = rec.tile([128, G, T], dt.bfloat16)
            bku = rec.tile([128, G, T], dt.uint16)
            bk32 = rec.tile([128, G, T], dt.float32)
            bksum = rec.tile([128, G], dt.float32)
            bsum = rec.tile([128, G], dt.float32)
            lnb = rec.tile([128, G, T], dt.float32)
            bb = BLKt[:].bitcast(dt.uint16)
            nc.vector.tensor_scalar(bm[:].bitcast(dt.uint16), bb,
                                    0x007F, 0x3F80, op0=AND, op1=OR)
            nc.vector.tensor_scalar(bku[:], bb, 7, None, op0=SHR)
            nc.vector.tensor_copy(bk32[:], bku[:])
            for g in range(G):
                nc.scalar.activation(lnb[:, g], bm[:, g], Ln,
                                     accum_out=bsum[:, g:g + 1])
            nc.vector.tensor_reduce(bksum[:], bk32[:], op=mybir.AluOpType.add,
                                    axis=mybir.AxisListType.X)

            # ln R = ln(mantissa) + k*ln2, with ln on [1,2) only
            Rb = Rt[:].bitcast(dt.uint32)
            kRu = scr.tile([128, G], dt.uint32, tag="kRu")
            nc.vector.tensor_scalar(kRu[:], Rb, 23, None, op0=SHR)
            kR = scr.tile([128, G], dt.float32, tag="kR")
            nc.vector.tensor_copy(kR[:], kRu[:])
            eR = scr.tile([128, G], dt.uint32, tag="eR")
            nc.vector.tensor_scalar(eR[:], Rb, 0x7F800000, None, op0=AND)
            riR = scr.tile([128, G], dt.float32, tag="riR")
            nc.vector.reciprocal(riR[:], eR[:].bitcast(dt.float32))
            mR = scr.tile([128, G], dt.float32, tag="mR")
            nc.vector.tensor_mul(mR[:], Rt[:], riR[:])
            lnR = scr.tile([128, G], dt.float32, tag="lnR")
            nc.scalar.activation(lnR[:], mR[:], Ln)
            # K = sum of all (k+127) exponents (exact integers in fp32)
            K = scr.tile([128, G], dt.float32, tag="K")
            nc.vector.tensor_add(K[:], logacc[:, 0::2], logacc[:, 1::2])
            nc.vector.tensor_add(K[:], K[:], kR[:])
            nc.vector.tensor_add(K[:], K[:], bksum[:])
            tot = scr.tile([128, G], dt.float32, tag="tot")
            nc.vector.tensor_scalar(tot[:], K[:], -KBIAS, LN2,
                                    op0=mybir.AluOpType.add,
                                    op1=mybir.AluOpType.mult)
            nc.vector.tensor_add(tot[:], tot[:], lnR[:])
            nc.vector.tensor_add(tot[:], tot[:], bsum[:])
            res = scr.tile([128, G], dt.float32, tag="res")
            nc.scalar.activation(res[:], tot[:], Copy, bias=CNORM, scale=-1.0)
            out_ap = bass.AP(loss_d.ap().tensor, 0, [[1, 128], [128, G]])
            nc.sync.dma_start(out_ap, res[:])

    nc.compile()
    _cache["nc"] = nc
    return nc


def _host_core(y, lab, c):
    """Inputs for core c: emission ratios, blanks, skip masks, bwd seed."""
    import ml_dtypes
    bf = ml_dtypes.bfloat16
    sl = slice(c * BPC, (c + 1) * BPC)
    yc = y[sl]
    labc = lab[sl].astype(np.int64)
    blank = yc[:, :, C - 1]                                  # [BPC,T]
    bs = np.arange(BPC)
    el = yc[bs[:, None, None], np.arange(T)[None, :, None],
            labc[:, None, :]]                                # [BPC,T,L]
    ratio = (el / blank[:, :, None]).astype(np.float32)
    r4 = ratio.reshape(G, 128, T, L)
    ELh = np.zeros((128, TP, SS, NB), np.float32)
    # fwd (even blocks): El_j(t) at slot 1+j, iterations 0..126
    ELh[:, 0:127, 1:17, 0::2] = r4[:, :, 0:127, :].transpose(1, 2, 3, 0)
    # bwd (odd blocks): El_{15-k}(254-i) at slot 1+k, iterations 0..127
    rr4 = ratio[:, ::-1, ::-1].reshape(G, 128, T, L)
    ELh[:, :, 1:17, 1::2] = rr4[:, :, 1:TP + 1, :].transpose(1, 2, 3, 0)
    BLKh = blank.reshape(G, 128, T).transpose(1, 0, 2)
    dd = np.zeros((BPC, L), np.float32)
    dd[:, 1:] = labc[:, 1:] != labc[:, :-1]
    ddg = dd.reshape(G, 128, L)
    DMh = np.zeros((128, SS, NB), np.float32)
    DMh[:, 1:16, 0::2] = ddg[:, :, 1:16].transpose(1, 2, 0)   # d_s
    DMh[:, 1:16, 1::2] = ddg[:, :, 15:0:-1].transpose(1, 2, 0)  # d_{16-s}
    IBh = ratio[:, T - 1, L - 1].reshape(G, 128).transpose(1, 0)
    return {
        "EL": np.ascontiguousarray(ELh.reshape(128, TP * SS * NB)).astype(bf),
        "BLK": np.ascontiguousarray(BLKh.reshape(128, G * T)).astype(bf),
        "DM": np.ascontiguousarray(DMh.reshape(128, SS * NB)).astype(bf),
        "IB": np.ascontiguousarray(IBh).astype(bf),
    }


def _fallback(y_pred, labels, input_length, label_length):
    """Exact log-domain numpy replica of the reference (generic lengths)."""
    y = np.asarray(y_pred, np.float32)
    lab = np.asarray(labels).astype(np.int64)
    il = np.asarray(input_length)[:, 0].astype(np.int64)
    ll = np.asarray(label_length)[:, 0].astype(np.int64)
    Bn, Tn, Cn = y.shape
    Ln = lab.shape[1]
    Sn = 2 * Ln + 1
    NEG = np.float32(-1e30)
    logp = np.log(y + 1e-7, dtype=np.float32)
    logp = logp - np.log(np.sum(np.exp(logp - logp.max(-1, keepdims=True)),
                                -1, keepdims=True)) - logp.max(-1, keepdims=True)
    ext = np.full((Bn, Sn), Cn - 1, np.int64)
    ext[:, 1::2] = lab
    sidx = np.arange(Sn)
    state_valid = sidx[None, :] < (2 * ll[:, None] + 1)
    skip = np.zeros((Bn, Sn), bool)
    skip[:, 3::2] = ext[:, 3::2] != ext[:, 1:-2:2]
    emit = logp[np.arange(Bn)[:, None, None], np.arange(Tn)[None, :, None],
                ext[:, None, :]]                      # [B,T,S]
    alpha = np.full((Bn, Sn), NEG, np.float32)
    alpha[:, 0] = emit[:, 0, 0]
    alpha[:, 1] = np.where(ll >= 1, emit[:, 0, 1], NEG)

    def lae(a, b):
        m = np.maximum(a, b)
        return m + np.log1p(np.exp(-np.abs(a - b)))
    for t in range(1, Tn):
        p1 = np.concatenate([np.full((Bn, 1), NEG), alpha[:, :-1]], 1)
        p2 = np.concatenate([np.full((Bn, 2), NEG), alpha[:, :-2]], 1)
        p2 = np.where(skip, p2, NEG)
        new = lae(lae(alpha, p1), p2) + emit[:, t, :]
        new = np.where(state_valid, new, NEG)
        alpha = np.where((t < il)[:, None], new, alpha)
    bi = np.arange(Bn)
    a_b = alpha[bi, 2 * ll]
    a_l = alpha[bi, np.maximum(2 * ll - 1, 0)]
    logp_f = np.where(ll > 0, lae(a_b, a_l), a_b)
    return (-logp_f[:, None]).astype(np.float32)


def _run(y_pred, labels, trace=False):
    from concourse import bass_utils
    nc = _build()
    y = np.asarray(y_pred, np.float32)
    lab = np.asarray(labels)
    in_maps = [_host_core(y, lab, c) for c in range(NCORES)]
    res = bass_utils.run_bass_kernel_spmd(nc, in_maps,
                                          core_ids=list(range(NCORES)),
                                          trace=trace)
    out = np.concatenate([res.results[c]["loss"] for c in range(NCORES)], 0)
    return out.astype(np.float32), res


def kernel(y_pred, labels, input_length, label_length):
    y_pred = np.ascontiguousarray(np.asarray(y_pred, np.float32))
    labels = np.asarray(labels)
    il = np.asarray(input_length)
    ll = np.asarray(label_length)
    if (y_pred.shape != (B, T, C) or labels.shape != (B, L)
            or not np.all(il == T) or not np.all(ll == L)):
        return _fallback(y_pred, labels, il, ll)

    try:
        out, _ = _run(y_pred, labels)
        return out
    except Exception:
        return _fallback(y_pred, labels, il, ll)
